# revision 4
# baseline (speedup 1.0000x reference)
"""Causal self-attention on 8 trn2 NeuronCores.

Sharding: core c -> (batch b = c//2, head-group g = c%2).  Each head-group
is 8 heads = 512 channels.  Per core:
  - q/k/v projections of x[b] restricted to the group's 512 columns
  - causal attention for the 8 heads, computed in the transposed
    orientation S^T = [tk, tq] so softmax denominators come from a
    ones-column appended to V (AV matmul yields them for free) and no
    transposes are needed anywhere
  - partial output projection through the group's 512 rows of Wo
Host sums the two partials per batch and adds (bv @ Wo + bo): softmax
weights sum to one, so the v-bias passes through attention additively.
"""

import numpy as np

import concourse.bass as bass
import concourse.mybir as mybir
from concourse import bacc, tile
from concourse.bass_utils import run_bass_kernel_spmd

B, T, C, H = 4, 2048, 1024, 16
HD = C // H          # 64
G = 2                # head groups (cores per batch)
HG = H // G          # 8 heads per group
CG = C // G          # 512 channels per group
P = 128
W = 512              # free-dim window (one PSUM bank of f32)
NW = T // W          # 4 windows
NTT = T // P         # 16 t tiles
NCI = C // P         # 8 c_in chunks
VS = HD + 1          # 65: v plus ones column

# "fp32" (exact, 4 cyc/row) or "f32r" (1 cyc/row, reduced precision)
MM_MODE = "f32r"

_cached_nc = None


def _build():
    f32 = mybir.dt.float32
    # storage dtype for every matmul operand: the BIR verifier requires f32r
    # matmul inputs to be *produced* as f32r, so the dtype is set at the
    # producer (DMA target / activation output), not bitcast at the matmul.
    mdt = mybir.dt.float32r if MM_MODE == "f32r" else f32
    AF = mybir.ActivationFunctionType
    nc = bacc.Bacc("TRN2", target_bir_lowering=False, debug=False, num_devices=8)

    xt_d = nc.dram_tensor("xt", [C, T], mdt, kind="ExternalInput")
    wq_d = nc.dram_tensor("wq", [C, CG], mdt, kind="ExternalInput")
    wk_d = nc.dram_tensor("wk", [C, CG], mdt, kind="ExternalInput")
    wv_d = nc.dram_tensor("wv", [C, CG], mdt, kind="ExternalInput")
    wo_d = nc.dram_tensor("wo", [CG, C], mdt, kind="ExternalInput")
    bq_d = nc.dram_tensor("bq", [P, CG // P], f32, kind="ExternalInput")
    bk_d = nc.dram_tensor("bk", [P, CG // P], f32, kind="ExternalInput")
    mask_d = nc.dram_tensor("mask", [P, P], f32, kind="ExternalInput")
    out_d = nc.dram_tensor("outp", [C, T], f32, kind="ExternalOutput")

    def mm(out, lhsT, rhs, start, stop):
        nc.tensor.matmul(out, lhsT, rhs, start=start, stop=stop)

    with tile.TileContext(nc) as tc:
        with (
            tc.tile_pool(name="pers", bufs=1) as pers,
            tc.tile_pool(name="psum_mm", bufs=3, space="PSUM") as psum_mm,
        ):
            qT = pers.tile([P, CG // P, T], mdt)    # q^T: [c_out, t]
            kT = pers.tile([P, CG // P, T], mdt)
            vp = pers.tile([P, NTT, HG * VS], mdt)  # v rows + ones col per head
            wo_sb = pers.tile([P, CG // P, C], mdt)
            mask_sb = pers.tile([P, P], f32)
            bq_sb = pers.tile([P, CG // P], f32)
            bk_sb = pers.tile([P, CG // P], f32)

            nc.sync.dma_start(out=wo_sb, in_=wo_d.ap().rearrange("(c p) n -> p c n", p=P))
            nc.sync.dma_start(out=mask_sb, in_=mask_d.ap())
            nc.sync.dma_start(out=bq_sb, in_=bq_d.ap())
            nc.sync.dma_start(out=bk_sb, in_=bk_d.ap())

            # ---- phase B: projections, two half-passes over c_in ----
            with (
                tc.tile_pool(name="xchunk", bufs=5) as xpool,
                tc.tile_pool(name="wchunk", bufs=2) as wpool,
            ):
                for half in range(2):
                    xc = []
                    for i in range(4):
                        ci = half * 4 + i
                        t_ = xpool.tile([P, T], mdt, tag="xc")
                        nc.sync.dma_start(out=t_, in_=xt_d.ap()[ci * P:(ci + 1) * P, :])
                        xc.append(t_)
                    wh = wpool.tile([P, 3, 4, CG], mdt)
                    for wi, wd in enumerate((wq_d, wk_d, wv_d)):
                        nc.sync.dma_start(
                            out=wh[:, wi, :, :],
                            in_=wd.ap()[half * CG:(half + 1) * CG, :].rearrange(
                                "(c p) n -> p c n", p=P
                            ),
                        )
                    # q^T and k^T: [c_out tile j, t window w]
                    for wi, (dst, b_sb) in enumerate(((qT, bq_sb), (kT, bk_sb))):
                        for j in range(CG // P):
                            for w in range(NW):
                                ps = psum_mm.tile([P, W], f32, tag="ps")
                                for i in range(4):
                                    mm(ps, wh[:, wi, i, j * P:(j + 1) * P],
                                       xc[i][:, w * W:(w + 1) * W],
                                       start=(i == 0), stop=(i == 3))
                                if half == 0:
                                    nc.scalar.activation(
                                        dst[:, j, w * W:(w + 1) * W], ps,
                                        AF.Identity, bias=b_sb[:, j:j + 1])
                                else:
                                    nc.vector.tensor_add(
                                        dst[:, j, w * W:(w + 1) * W],
                                        dst[:, j, w * W:(w + 1) * W], ps)
                    # v in row layout [t, c_out], strided into vp per head
                    for it in range(NTT):
                        ps = psum_mm.tile([P, CG], f32, tag="ps")
                        for i in range(4):
                            mm(ps, xc[i][:, it * P:(it + 1) * P], wh[:, 2, i, :],
                               start=(i == 0), stop=(i == 3))
                        v_view = vp[:, it, :].rearrange("p (h x) -> p h x", x=VS)[:, :, 0:HD]
                        ps_view = ps.rearrange("p (h x) -> p h x", x=HD)
                        if half == 0:
                            nc.scalar.copy(v_view, ps_view)
                        else:
                            nc.vector.tensor_add(v_view, v_view, ps_view)
            ones_sb = pers.tile([P, HG], f32)
            nc.vector.memset(ones_sb, 1.0)
            for it in range(NTT):
                ones_view = vp[:, it, :].rearrange("p (h x) -> p h x", x=VS)[:, :, HD:VS]
                nc.vector.tensor_copy(
                    ones_view, ones_sb.rearrange("p (h x) -> p h x", x=1))

            # ---- phase C: attention ----
            with (
                tc.tile_pool(name="attn", bufs=1) as attn,
                tc.tile_pool(name="pt", bufs=4) as ptpool,
                tc.tile_pool(name="rc", bufs=2) as rcpool,
                tc.tile_pool(name="rb", bufs=2) as rbpool,
                tc.tile_pool(name="psum_av", bufs=2, space="PSUM") as psum_av,
                tc.tile_pool(name="osb", bufs=3) as opool,
                tc.tile_pool(name="psum_o", bufs=2, space="PSUM") as psum_o,
            ):
                yT = attn.tile([P, CG // P, T], mdt)
                for h in range(HG):
                    hp, ho = h // 2, (h % 2) * HD
                    for w in range(NW):
                        tq0 = w * W
                        ps_av = psum_av.tile([VS, W], f32, tag="av")
                        ntk = (w + 1) * (W // P)
                        for i in range(ntk):
                            tk0 = i * P
                            vs = max(tq0, tk0)
                            n = tq0 + W - vs
                            ps_s = psum_mm.tile([P, W], f32, tag="ps")
                            # S^T tile: [tk, tq] = k_h^T(stat) x q_h(move)
                            mm(ps_s[:, :n], kT[ho:ho + HD, hp, tk0:tk0 + P],
                               qT[ho:ho + HD, hp, vs:vs + n], start=True, stop=True)
                            pt = ptpool.tile([P, W], mdt, tag="pt")
                            nc.scalar.activation(pt[:, :n], ps_s[:, :n], AF.Exp,
                                                 scale=1.0 / float(np.sqrt(HD)))
                            if tk0 >= tq0:  # diagonal tile: zero tq < tk
                                nc.vector.tensor_mul(pt[:, :P], pt[:, :P], mask_sb)
                            mm(ps_av[:, vs - tq0:], vp[:, i, h * VS:(h + 1) * VS],
                               pt[:, :n], start=(i == 0), stop=(i == ntk - 1))
                        rc = rcpool.tile([1, W], f32, tag="rc")
                        nc.vector.reciprocal(rc, ps_av[HD:VS, :])
                        rb = rbpool.tile([HD, W], f32, tag="rb")
                        nc.gpsimd.partition_broadcast(rb, rc)
                        nc.vector.tensor_mul(
                            yT[ho:ho + HD, hp, tq0:tq0 + W], ps_av[0:HD, :], rb)

                # ---- phase D: partial output projection ----
                for w in range(NW):
                    for m in range(C // P):
                        po = psum_o.tile([P, W], f32, tag="po")
                        for i in range(CG // P):
                            mm(po, wo_sb[:, i, m * P:(m + 1) * P],
                               yT[:, i, w * W:(w + 1) * W],
                               start=(i == 0), stop=(i == CG // P - 1))
                        ot = opool.tile([P, W], f32, tag="ot")
                        nc.scalar.copy(ot, po)
                        nc.sync.dma_start(
                            out=out_d.ap()[m * P:(m + 1) * P, w * W:(w + 1) * W],
                            in_=ot)

    nc.compile()
    return nc


def get_nc():
    global _cached_nc
    if _cached_nc is None:
        _cached_nc = _build()
    return _cached_nc


def make_in_maps(x, Wq, bq, Wk, bk, Wv, bv, Wo, bo):
    x = np.asarray(x, np.float32)
    mask = np.triu(np.ones((P, P), np.float32))
    in_maps = []
    for c in range(8):
        b, g = c // 2, c % 2
        cs = slice(g * CG, (g + 1) * CG)
        in_maps.append({
            "xt": np.ascontiguousarray(x[b].T),
            "wq": np.ascontiguousarray(np.asarray(Wq, np.float32)[:, cs]),
            "wk": np.ascontiguousarray(np.asarray(Wk, np.float32)[:, cs]),
            "wv": np.ascontiguousarray(np.asarray(Wv, np.float32)[:, cs]),
            "wo": np.ascontiguousarray(np.asarray(Wo, np.float32)[cs, :]),
            "bq": np.ascontiguousarray(
                np.asarray(bq, np.float32)[cs].reshape(CG // P, P).T),
            "bk": np.ascontiguousarray(
                np.asarray(bk, np.float32)[cs].reshape(CG // P, P).T),
            "mask": mask,
        })
    return in_maps


def combine(results, Wv, bv, Wo, bo):
    const = (np.asarray(bv, np.float32) @ np.asarray(Wo, np.float32)
             + np.asarray(bo, np.float32))
    out = np.empty((B, T, C), np.float32)
    for b in range(B):
        acc = results[2 * b]["outp"] + results[2 * b + 1]["outp"]
        out[b] = acc.T + const[None, :]
    return out


def kernel(x, Wq, bq, Wk, bk, Wv, bv, Wo, bo):
    nc = get_nc()
    in_maps = make_in_maps(x, Wq, bq, Wk, bk, Wv, bv, Wo, bo)
    res = run_bass_kernel_spmd(nc, in_maps, core_ids=list(range(8)))
    return combine(res.results, Wv, bv, Wo, bo)


# revision 6
# speedup vs baseline: 1.1057x; 1.1057x over previous
"""Causal self-attention on 8 trn2 NeuronCores.

Sharding: core c -> (batch b = c//2, head-group g = c%2).  Each head-group
is 8 heads = 512 channels.  Per core:
  - q/k/v projections of x[b] restricted to the group's 512 columns
  - causal attention for the 8 heads, computed in the transposed
    orientation S^T = [tk, tq] so softmax denominators come from a
    ones-column appended to V (AV matmul yields them for free) and no
    transposes are needed anywhere
  - partial output projection through the group's 512 rows of Wo
Host sums the two partials per batch and adds (bv @ Wo + bo): softmax
weights sum to one, so the v-bias passes through attention additively.

The QK stationary operand is zero-padded to a full [128,128] footprint
(kz holds each head's k tile in its 64 q-channel rows, zeros in the other
head's rows, so the full 128-partition q pair streams through) — without
this the PE activity monitor keeps the tensor engine clock-gated at
1.2 GHz for the whole attention phase.
"""

import numpy as np
import ml_dtypes

import concourse.bass as bass
import concourse.mybir as mybir
from concourse import bacc, tile
from concourse.bass_utils import run_bass_kernel_spmd

B, T, C, H = 4, 2048, 1024, 16
HD = C // H          # 64
G = 2                # head groups (cores per batch)
HG = H // G          # 8 heads per group
CG = C // G          # 512 channels per group
CGP = CG // 128      # 4 c_out tiles per group
P = 128
W = 512              # free-dim window (one PSUM bank of f32)
NW = T // W          # 4 windows
NTT = T // P         # 16 t tiles
VS = HD + 1          # 65: v plus ones column

_cached_nc = None


def _build():
    f32 = mybir.dt.float32
    f32r = mybir.dt.float32r
    bf16 = mybir.dt.bfloat16
    AF = mybir.ActivationFunctionType
    nc = bacc.Bacc("TRN2", target_bir_lowering=False, debug=False, num_devices=8)

    xt_d = nc.dram_tensor("xt", [C, T], f32r, kind="ExternalInput")
    wq_d = nc.dram_tensor("wq", [C, CG], f32r, kind="ExternalInput")
    wk_d = nc.dram_tensor("wk", [C, CG], f32r, kind="ExternalInput")
    wv_d = nc.dram_tensor("wv", [C, CG], f32r, kind="ExternalInput")
    wo_d = nc.dram_tensor("wo", [CG, C], f32r, kind="ExternalInput")
    bq_d = nc.dram_tensor("bq", [P, CGP], f32, kind="ExternalInput")
    bk_d = nc.dram_tensor("bk", [P, CGP], f32, kind="ExternalInput")
    mask_d = nc.dram_tensor("mask", [P, P], f32, kind="ExternalInput")
    out_d = nc.dram_tensor("outp", [C, T], f32, kind="ExternalOutput")

    mm = lambda out, lhsT, rhs, start, stop: nc.tensor.matmul(
        out, lhsT, rhs, start=start, stop=stop)

    with tile.TileContext(nc) as tc:
        with (
            tc.tile_pool(name="pers", bufs=1) as pers,
            tc.tile_pool(name="psum_mm", bufs=2, space="PSUM") as psum_mm,
        ):
            qT = pers.tile([P, CGP, T], bf16)        # q^T: [c_out, t]
            # kz[:, j, a, tk]: head h=2j+a k-tile in its own 64 rows, 0 else
            kz = pers.tile([P, CGP, 2, T], bf16)
            vp = pers.tile([P, NTT, HG * VS], f32r)  # v rows + ones col/head
            wo_sb = pers.tile([P, CGP, C], f32r)
            mask_sb = pers.tile([P, P], f32)
            bq_sb = pers.tile([P, CGP], f32)
            bk_sb = pers.tile([P, CGP], f32)
            ones_sb = pers.tile([P, HG], f32)

            nc.sync.dma_start(out=wo_sb, in_=wo_d.ap().rearrange("(c p) n -> p c n", p=P))
            nc.sync.dma_start(out=mask_sb, in_=mask_d.ap())
            nc.sync.dma_start(out=bq_sb, in_=bq_d.ap())
            nc.sync.dma_start(out=bk_sb, in_=bk_d.ap())
            nc.vector.memset(ones_sb, 1.0)
            # zero halves of kz that the k-projection copies never touch
            nc.vector.memset(kz[0:HD, :, 1, :], 0.0)
            nc.vector.memset(kz[HD:P, :, 0, :], 0.0)

            # ---- phase B: projections, two half-passes over c_in ----
            with (
                tc.tile_pool(name="xchunk", bufs=6) as xpool,
                tc.tile_pool(name="wchunk", bufs=2) as wpool,
            ):
                for half in range(2):
                    xc = []
                    for i in range(4):
                        ci = half * 4 + i
                        t_ = xpool.tile([P, T], f32r, tag="xc")
                        nc.sync.dma_start(out=t_, in_=xt_d.ap()[ci * P:(ci + 1) * P, :])
                        xc.append(t_)
                    wh = wpool.tile([P, 3, 4, CG], f32r)
                    for wi, wd in enumerate((wq_d, wk_d, wv_d)):
                        nc.sync.dma_start(
                            out=wh[:, wi, :, :],
                            in_=wd.ap()[half * CG:(half + 1) * CG, :].rearrange(
                                "(c p) n -> p c n", p=P
                            ),
                        )
                    for j in range(CGP):
                        for w in range(NW):
                            ws = slice(w * W, (w + 1) * W)
                            # q
                            ps = psum_mm.tile([P, W], f32, tag="ps")
                            for i in range(4):
                                mm(ps, wh[:, 0, i, j * P:(j + 1) * P],
                                   xc[i][:, ws], start=(i == 0), stop=(i == 3))
                            if half == 0:
                                nc.scalar.activation(qT[:, j, ws], ps,
                                                     AF.Identity, bias=bq_sb[:, j:j + 1])
                            else:
                                nc.vector.tensor_add(qT[:, j, ws], qT[:, j, ws], ps)
                            # k -> zero-padded kz layout
                            ps = psum_mm.tile([P, W], f32, tag="ps")
                            for i in range(4):
                                mm(ps, wh[:, 1, i, j * P:(j + 1) * P],
                                   xc[i][:, ws], start=(i == 0), stop=(i == 3))
                            for a in range(2):
                                rows = slice(a * HD, (a + 1) * HD)
                                dst = kz[rows, j, a, ws]
                                if half == 0:
                                    nc.scalar.activation(dst, ps[rows, :], AF.Identity,
                                                         bias=bk_sb[rows, j:j + 1])
                                else:
                                    nc.vector.tensor_add(dst, dst, ps[rows, :])
                    # v in row layout [t, c_out], strided into vp per head
                    for it in range(NTT):
                        ps = psum_mm.tile([P, CG], f32, tag="ps")
                        for i in range(4):
                            mm(ps, xc[i][:, it * P:(it + 1) * P], wh[:, 2, i, :],
                               start=(i == 0), stop=(i == 3))
                        v_view = vp[:, it, :].rearrange("p (h x) -> p h x", x=VS)[:, :, 0:HD]
                        ps_view = ps.rearrange("p (h x) -> p h x", x=HD)
                        if half == 0:
                            nc.scalar.copy(v_view, ps_view)
                        else:
                            nc.vector.tensor_add(v_view, v_view, ps_view)
            for it in range(NTT):
                ones_view = vp[:, it, :].rearrange("p (h x) -> p h x", x=VS)[:, :, HD:VS]
                nc.vector.tensor_copy(
                    ones_view, ones_sb.rearrange("p (h x) -> p h x", x=1))

            # ---- phase C+D: attention with interleaved output projection ----
            with (
                tc.tile_pool(name="attn", bufs=1) as attn,
                tc.tile_pool(name="pt", bufs=4) as ptpool,
                tc.tile_pool(name="dn", bufs=2) as dnpool,
                tc.tile_pool(name="rb", bufs=2) as rbpool,
                tc.tile_pool(name="psum_av", bufs=2, space="PSUM") as psum_av,
                tc.tile_pool(name="osb", bufs=3) as opool,
                tc.tile_pool(name="psum_o", bufs=2, space="PSUM") as psum_o,
            ):
                yT = attn.tile([P, CGP, T], f32r)
                for w in range(NW):
                    tq0 = w * W
                    for h in range(HG):
                        j, a = h // 2, h % 2
                        ps_av = psum_av.tile([VS, W], f32, tag="av")
                        ntk = (w + 1) * (W // P)
                        # full-width tk blocks (below the diagonal) pair up to
                        # share one psum pair-tile and one exp; the 4 diagonal
                        # blocks are processed individually
                        nfull = tq0 // P
                        groups = [[x, x + 1] for x in range(0, nfull, 2)]
                        groups += [[x] for x in range(nfull, ntk)]
                        escale = 1.0 / float(np.sqrt(HD))
                        for grp in groups:
                            ps_s = psum_mm.tile([P, 2, W], f32, tag="ps")
                            pt = ptpool.tile([P, 2, W], f32r, tag="pt")
                            for u, i in enumerate(grp):
                                vs = max(tq0, i * P)
                                n = tq0 + W - vs
                                mm(ps_s[:, u, W - n:], kz[:, j, a, i * P:(i + 1) * P],
                                   qT[:, j, vs:vs + n], start=True, stop=True)
                            if len(grp) == 2:
                                nc.scalar.activation(pt, ps_s, AF.Exp, scale=escale)
                            else:
                                n = tq0 + W - max(tq0, grp[0] * P)
                                nc.scalar.activation(pt[:, 0, W - n:],
                                                     ps_s[:, 0, W - n:],
                                                     AF.Exp, scale=escale)
                            for u, i in enumerate(grp):
                                tk0 = i * P
                                vs = max(tq0, tk0)
                                n = tq0 + W - vs
                                if tk0 >= tq0:  # diagonal block: zero tq < tk
                                    nc.vector.tensor_mul(
                                        pt[:, u, W - n:W - n + P],
                                        pt[:, u, W - n:W - n + P], mask_sb)
                                mm(ps_av[:, vs - tq0:], vp[:, i, h * VS:(h + 1) * VS],
                                   pt[:, u, W - n:], start=(i == 0), stop=(i == ntk - 1))
                        dn = dnpool.tile([1, W], f32, tag="dn")
                        nc.scalar.copy(dn, ps_av[HD:VS, :])
                        rb = rbpool.tile([HD, W], f32, tag="rb")
                        nc.gpsimd.partition_broadcast(rb, dn)
                        nc.vector.reciprocal(rb, rb)
                        nc.vector.tensor_mul(
                            yT[a * HD:(a + 1) * HD, j, tq0:tq0 + W],
                            ps_av[0:HD, :], rb)
                    # output projection for this window
                    for m in range(C // P):
                        po = psum_o.tile([P, W], f32, tag="po")
                        for i in range(CGP):
                            mm(po, wo_sb[:, i, m * P:(m + 1) * P],
                               yT[:, i, tq0:tq0 + W],
                               start=(i == 0), stop=(i == CGP - 1))
                        ot = opool.tile([P, W], f32, tag="ot")
                        nc.vector.tensor_copy(ot, po)
                        nc.sync.dma_start(
                            out=out_d.ap()[m * P:(m + 1) * P, tq0:tq0 + W],
                            in_=ot)

    nc.compile()
    return nc


def get_nc():
    global _cached_nc
    if _cached_nc is None:
        _cached_nc = _build()
    return _cached_nc


def make_in_maps(x, Wq, bq, Wk, bk, Wv, bv, Wo, bo):
    x = np.asarray(x, np.float32)
    mask = np.triu(np.ones((P, P), np.float32))
    in_maps = []
    for c in range(8):
        b, g = c // 2, c % 2
        cs = slice(g * CG, (g + 1) * CG)
        in_maps.append({
            "xt": np.ascontiguousarray(x[b].T),
            "wq": np.ascontiguousarray(np.asarray(Wq, np.float32)[:, cs]),
            "wk": np.ascontiguousarray(np.asarray(Wk, np.float32)[:, cs]),
            "wv": np.ascontiguousarray(np.asarray(Wv, np.float32)[:, cs]),
            "wo": np.ascontiguousarray(np.asarray(Wo, np.float32)[cs, :]),
            "bq": np.ascontiguousarray(
                np.asarray(bq, np.float32)[cs].reshape(CGP, P).T),
            "bk": np.ascontiguousarray(
                np.asarray(bk, np.float32)[cs].reshape(CGP, P).T),
            "mask": mask,
        })
    return in_maps


def combine(results, Wv, bv, Wo, bo):
    const = (np.asarray(bv, np.float32) @ np.asarray(Wo, np.float32)
             + np.asarray(bo, np.float32))
    out = np.empty((B, T, C), np.float32)
    for b in range(B):
        acc = results[2 * b]["outp"] + results[2 * b + 1]["outp"]
        out[b] = acc.T + const[None, :]
    return out


def kernel(x, Wq, bq, Wk, bk, Wv, bv, Wo, bo):
    nc = get_nc()
    in_maps = make_in_maps(x, Wq, bq, Wk, bk, Wv, bv, Wo, bo)
    res = run_bass_kernel_spmd(nc, in_maps, core_ids=list(range(8)))
    return combine(res.results, Wv, bv, Wo, bo)


# revision 7
# speedup vs baseline: 1.3037x; 1.1791x over previous
"""Causal self-attention on 8 trn2 NeuronCores.

Sharding: core c -> (batch b = c//2, head-group g = c%2).  Each head-group
is 8 heads = 512 channels.  Per core:
  - q/k/v projections of x[b] restricted to the group's 512 columns
  - causal attention for the 8 heads, computed in the transposed
    orientation S^T = [tk, tq] so softmax denominators come from a
    ones-column appended to V (AV matmul yields them for free) and no
    transposes are needed anywhere
  - partial output projection through the group's 512 rows of Wo
Host sums the two partials per batch and adds (bv @ Wo + bo): softmax
weights sum to one, so the v-bias passes through attention additively.

The QK stationary operand is zero-padded to a full [128,128] footprint
(kz holds each head's k tile in its 64 q-channel rows, zeros in the other
head's rows, so the full 128-partition q pair streams through) — without
this the PE activity monitor keeps the tensor engine clock-gated at
1.2 GHz for the whole attention phase.
"""

import numpy as np
import ml_dtypes

import concourse.bass as bass
import concourse.mybir as mybir
from concourse import bacc, tile
from concourse.bass_utils import run_bass_kernel_spmd

B, T, C, H = 4, 2048, 1024, 16
HD = C // H          # 64
G = 2                # head groups (cores per batch)
HG = H // G          # 8 heads per group
CG = C // G          # 512 channels per group
CGP = CG // 128      # 4 c_out tiles per group
P = 128
W = 512              # free-dim window (one PSUM bank of f32)
NW = T // W          # 4 windows
NTT = T // P         # 16 t tiles
VS = HD + 1          # 65: v plus ones column

_cached_nc = None


def _build():
    f32 = mybir.dt.float32
    f32r = mybir.dt.float32r
    bf16 = mybir.dt.bfloat16
    AF = mybir.ActivationFunctionType
    nc = bacc.Bacc("TRN2", target_bir_lowering=False, debug=False, num_devices=8)

    xt_d = nc.dram_tensor("xt", [C, T], f32r, kind="ExternalInput")
    wq_d = nc.dram_tensor("wq", [C, CG], f32r, kind="ExternalInput")
    wk_d = nc.dram_tensor("wk", [C, CG], f32r, kind="ExternalInput")
    wv_d = nc.dram_tensor("wv", [C, CG], f32r, kind="ExternalInput")
    wo_d = nc.dram_tensor("wo", [CG, C], f32r, kind="ExternalInput")
    bq_d = nc.dram_tensor("bq", [P, CGP], f32, kind="ExternalInput")
    bk_d = nc.dram_tensor("bk", [P, CGP], f32, kind="ExternalInput")
    mask_d = nc.dram_tensor("mask", [P, P], f32, kind="ExternalInput")
    out_d = nc.dram_tensor("outp", [C, T], f32, kind="ExternalOutput")

    mm = lambda out, lhsT, rhs, start, stop: nc.tensor.matmul(
        out, lhsT, rhs, start=start, stop=stop)

    with tile.TileContext(nc) as tc:
        with (
            tc.tile_pool(name="pers", bufs=1) as pers,
            tc.tile_pool(name="psum_mm", bufs=2, space="PSUM") as psum_mm,
        ):
            qT = pers.tile([P, CGP, T], bf16)        # q^T: [c_out, t]
            # kz[:, j, a, tk]: head h=2j+a k-tile in its own 64 rows, 0 else
            kz = pers.tile([P, CGP, 2, T], bf16)
            vp = pers.tile([P, NTT, HG * VS], f32r)  # v rows + ones col/head
            wo_sb = pers.tile([P, CGP, C], f32r)
            mask_sb = pers.tile([P, P], f32)
            bq_sb = pers.tile([P, CGP], f32)
            bk_sb = pers.tile([P, CGP], f32)
            ones_sb = pers.tile([P, HG], f32)

            nc.sync.dma_start(out=wo_sb, in_=wo_d.ap().rearrange("(c p) n -> p c n", p=P))
            nc.sync.dma_start(out=mask_sb, in_=mask_d.ap())
            nc.sync.dma_start(out=bq_sb, in_=bq_d.ap())
            nc.sync.dma_start(out=bk_sb, in_=bk_d.ap())
            nc.vector.memset(ones_sb, 1.0)
            # zero halves of kz that the k-projection copies never touch
            nc.vector.memset(kz[0:HD, :, 1, :], 0.0)
            nc.vector.memset(kz[HD:P, :, 0, :], 0.0)

            # ---- phase B: projections, two half-passes over c_in ----
            with (
                tc.tile_pool(name="xchunk", bufs=6) as xpool,
                tc.tile_pool(name="wchunk", bufs=2) as wpool,
            ):
                for half in range(2):
                    xc = []
                    for i in range(4):
                        ci = half * 4 + i
                        t_ = xpool.tile([P, T], f32r, tag="xc")
                        nc.sync.dma_start(out=t_, in_=xt_d.ap()[ci * P:(ci + 1) * P, :])
                        xc.append(t_)
                    wh = wpool.tile([P, 3, 4, CG], f32r)
                    for wi, wd in enumerate((wq_d, wk_d, wv_d)):
                        nc.sync.dma_start(
                            out=wh[:, wi, :, :],
                            in_=wd.ap()[half * CG:(half + 1) * CG, :].rearrange(
                                "(c p) n -> p c n", p=P
                            ),
                        )
                    for j in range(CGP):
                        for w in range(NW):
                            ws = slice(w * W, (w + 1) * W)
                            # q
                            ps = psum_mm.tile([P, W], f32, tag="ps")
                            for i in range(4):
                                mm(ps, wh[:, 0, i, j * P:(j + 1) * P],
                                   xc[i][:, ws], start=(i == 0), stop=(i == 3))
                            if half == 0:
                                nc.scalar.activation(qT[:, j, ws], ps,
                                                     AF.Identity, bias=bq_sb[:, j:j + 1])
                            else:
                                nc.vector.tensor_add(qT[:, j, ws], qT[:, j, ws], ps)
                            # k -> zero-padded kz layout
                            ps = psum_mm.tile([P, W], f32, tag="ps")
                            for i in range(4):
                                mm(ps, wh[:, 1, i, j * P:(j + 1) * P],
                                   xc[i][:, ws], start=(i == 0), stop=(i == 3))
                            for a in range(2):
                                rows = slice(a * HD, (a + 1) * HD)
                                dst = kz[rows, j, a, ws]
                                if half == 0:
                                    nc.scalar.activation(dst, ps[rows, :], AF.Identity,
                                                         bias=bk_sb[rows, j:j + 1])
                                else:
                                    nc.vector.tensor_add(dst, dst, ps[rows, :])
                    # v in row layout [t, c_out], strided into vp per head
                    for it in range(NTT):
                        ps = psum_mm.tile([P, CG], f32, tag="ps")
                        for i in range(4):
                            mm(ps, xc[i][:, it * P:(it + 1) * P], wh[:, 2, i, :],
                               start=(i == 0), stop=(i == 3))
                        v_view = vp[:, it, :].rearrange("p (h x) -> p h x", x=VS)[:, :, 0:HD]
                        ps_view = ps.rearrange("p (h x) -> p h x", x=HD)
                        if half == 0:
                            nc.scalar.copy(v_view, ps_view)
                        else:
                            nc.vector.tensor_add(v_view, v_view, ps_view)
            for it in range(NTT):
                ones_view = vp[:, it, :].rearrange("p (h x) -> p h x", x=VS)[:, :, HD:VS]
                nc.vector.tensor_copy(
                    ones_view, ones_sb.rearrange("p (h x) -> p h x", x=1))

            # ---- phase C+D: attention with interleaved output projection ----
            with (
                tc.tile_pool(name="attn", bufs=1) as attn,
                tc.tile_pool(name="pt", bufs=6) as ptpool,
                tc.tile_pool(name="dn", bufs=2) as dnpool,
                tc.tile_pool(name="rb", bufs=2) as rbpool,
                tc.tile_pool(name="psum_av", bufs=3, space="PSUM") as psum_av,
                tc.tile_pool(name="osb", bufs=3) as opool,
                tc.tile_pool(name="psum_o", bufs=1, space="PSUM") as psum_o,
            ):
                yT = attn.tile([P, CGP, T], f32r)
                for w in range(NW):
                    tq0 = w * W
                    for h in range(HG):
                        j, a = h // 2, h % 2
                        ps_av = psum_av.tile([VS, W], f32, tag="av")
                        ntk = (w + 1) * (W // P)
                        # full-width tk blocks (below the diagonal) pair up to
                        # share one psum pair-tile and one exp; the 4 diagonal
                        # blocks are processed individually
                        nfull = tq0 // P
                        groups = [[x, x + 1] for x in range(0, nfull, 2)]
                        groups += [[x] for x in range(nfull, ntk)]
                        escale = 1.0 / float(np.sqrt(HD))
                        for grp in groups:
                            ps_s = psum_mm.tile([P, 2, W], f32, tag="ps")
                            pt = ptpool.tile([P, 2, W], f32r, tag="pt")
                            for u, i in enumerate(grp):
                                vs = max(tq0, i * P)
                                n = tq0 + W - vs
                                mm(ps_s[:, u, W - n:], kz[:, j, a, i * P:(i + 1) * P],
                                   qT[:, j, vs:vs + n], start=True, stop=True)
                            if len(grp) == 2:
                                nc.scalar.activation(pt, ps_s, AF.Exp, scale=escale)
                            else:
                                n = tq0 + W - max(tq0, grp[0] * P)
                                nc.scalar.activation(pt[:, 0, W - n:],
                                                     ps_s[:, 0, W - n:],
                                                     AF.Exp, scale=escale)
                            for u, i in enumerate(grp):
                                tk0 = i * P
                                vs = max(tq0, tk0)
                                n = tq0 + W - vs
                                if tk0 >= tq0:  # diagonal block: zero tq < tk
                                    nc.vector.tensor_mul(
                                        pt[:, u, W - n:W - n + P],
                                        pt[:, u, W - n:W - n + P], mask_sb)
                                mm(ps_av[:, vs - tq0:], vp[:, i, h * VS:(h + 1) * VS],
                                   pt[:, u, W - n:], start=(i == 0), stop=(i == ntk - 1))
                        dn = dnpool.tile([1, W], f32, tag="dn")
                        nc.scalar.copy(dn, ps_av[HD:VS, :])
                        rb = rbpool.tile([HD, W], f32, tag="rb")
                        nc.gpsimd.partition_broadcast(rb, dn)
                        nc.vector.reciprocal_approx_fast(out=rb, in_=rb)
                        nc.vector.tensor_mul(
                            yT[a * HD:(a + 1) * HD, j, tq0:tq0 + W],
                            ps_av[0:HD, :], rb)
                    # output projection for this window
                    for m in range(C // P):
                        po = psum_o.tile([P, W], f32, tag="po")
                        for i in range(CGP):
                            mm(po, wo_sb[:, i, m * P:(m + 1) * P],
                               yT[:, i, tq0:tq0 + W],
                               start=(i == 0), stop=(i == CGP - 1))
                        ot = opool.tile([P, W], f32, tag="ot")
                        nc.vector.tensor_copy(ot, po)
                        nc.sync.dma_start(
                            out=out_d.ap()[m * P:(m + 1) * P, tq0:tq0 + W],
                            in_=ot)

    nc.compile()
    return nc


def get_nc():
    global _cached_nc
    if _cached_nc is None:
        _cached_nc = _build()
    return _cached_nc


def make_in_maps(x, Wq, bq, Wk, bk, Wv, bv, Wo, bo):
    x = np.asarray(x, np.float32)
    mask = np.triu(np.ones((P, P), np.float32))
    in_maps = []
    for c in range(8):
        b, g = c // 2, c % 2
        cs = slice(g * CG, (g + 1) * CG)
        in_maps.append({
            "xt": np.ascontiguousarray(x[b].T),
            "wq": np.ascontiguousarray(np.asarray(Wq, np.float32)[:, cs]),
            "wk": np.ascontiguousarray(np.asarray(Wk, np.float32)[:, cs]),
            "wv": np.ascontiguousarray(np.asarray(Wv, np.float32)[:, cs]),
            "wo": np.ascontiguousarray(np.asarray(Wo, np.float32)[cs, :]),
            "bq": np.ascontiguousarray(
                np.asarray(bq, np.float32)[cs].reshape(CGP, P).T),
            "bk": np.ascontiguousarray(
                np.asarray(bk, np.float32)[cs].reshape(CGP, P).T),
            "mask": mask,
        })
    return in_maps


def combine(results, Wv, bv, Wo, bo):
    const = (np.asarray(bv, np.float32) @ np.asarray(Wo, np.float32)
             + np.asarray(bo, np.float32))
    out = np.empty((B, T, C), np.float32)
    for b in range(B):
        acc = results[2 * b]["outp"] + results[2 * b + 1]["outp"]
        out[b] = acc.T + const[None, :]
    return out


def kernel(x, Wq, bq, Wk, bk, Wv, bv, Wo, bo):
    nc = get_nc()
    in_maps = make_in_maps(x, Wq, bq, Wk, bk, Wv, bv, Wo, bo)
    res = run_bass_kernel_spmd(nc, in_maps, core_ids=list(range(8)))
    return combine(res.results, Wv, bv, Wo, bo)


# revision 14
# speedup vs baseline: 1.3777x; 1.0568x over previous
"""Causal self-attention on 8 trn2 NeuronCores.

Sharding: core c -> (batch b = c//2, head-group g = c%2).  Each head-group
is 8 heads = 512 channels.  Per core:
  - q/k/v projections of x[b] restricted to the group's 512 columns
  - causal attention for the 8 heads, computed in the transposed
    orientation S^T = [tk, tq] so softmax denominators come from a
    ones-column appended to V (AV matmul yields them for free) and no
    transposes are needed anywhere
  - partial output projection through the group's 512 rows of Wo
Host sums the two partials per batch and adds (bv @ Wo + bo): softmax
weights sum to one, so the v-bias passes through attention additively.

The QK stationary operand is zero-padded to a full [128,128] footprint
(kz holds each head's k tile in its 64 q-channel rows, zeros in the other
head's rows, so the full 128-partition q pair streams through) — without
this the PE activity monitor keeps the tensor engine clock-gated at
1.2 GHz for the whole attention phase.
"""

import numpy as np
import ml_dtypes

import concourse.bass as bass
import concourse.mybir as mybir
from concourse import bacc, tile
from concourse.bass_utils import run_bass_kernel_spmd

B, T, C, H = 4, 2048, 1024, 16
HD = C // H          # 64
G = 2                # head groups (cores per batch)
HG = H // G          # 8 heads per group
CG = C // G          # 512 channels per group
CGP = CG // 128      # 4 c_out tiles per group
P = 128
W = 512              # free-dim window (one PSUM bank of f32)
NW = T // W          # 4 windows
NTT = T // P         # 16 t tiles
VS = HD + 1          # 65: v plus ones column

_cached_nc = None


def _build():
    f32 = mybir.dt.float32
    f32r = mybir.dt.float32r
    bf16 = mybir.dt.bfloat16
    AF = mybir.ActivationFunctionType
    nc = bacc.Bacc("TRN2", target_bir_lowering=False, debug=False, num_devices=8)

    xt_d = nc.dram_tensor("xt", [C, T], f32r, kind="ExternalInput")
    wq_d = nc.dram_tensor("wq", [C, CG], f32r, kind="ExternalInput")
    wk_d = nc.dram_tensor("wk", [C, CG], f32r, kind="ExternalInput")
    wv_d = nc.dram_tensor("wv", [C, CG], f32r, kind="ExternalInput")
    wo_d = nc.dram_tensor("wo", [CG, C], f32r, kind="ExternalInput")
    bq_d = nc.dram_tensor("bq", [P, CGP], f32, kind="ExternalInput")
    bk_d = nc.dram_tensor("bk", [P, CGP], f32, kind="ExternalInput")
    mask_d = nc.dram_tensor("mask", [P, P], f32, kind="ExternalInput")
    mask2_d = nc.dram_tensor("mask2", [P, 2 * P], f32, kind="ExternalInput")
    out_d = nc.dram_tensor("outp", [C, T], f32, kind="ExternalOutput")

    mm = lambda out, lhsT, rhs, start, stop: nc.tensor.matmul(
        out, lhsT, rhs, start=start, stop=stop)

    with tile.TileContext(nc) as tc:
        with (
            tc.tile_pool(name="pers", bufs=1) as pers,
            tc.tile_pool(name="psum_mm", bufs=2, space="PSUM") as psum_mm,
        ):
            qT = pers.tile([P, CGP, T], bf16)        # q^T: [c_out, t]
            # kz[:, j, a, tk]: head h=2j+a k-tile in its own 64 rows, 0 else
            kz = pers.tile([P, CGP, 2, T], bf16)
            vp = pers.tile([P, NTT, HG * VS], f32r)  # v rows + ones col/head
            wo_sb = pers.tile([P, CGP, C], f32r)
            mask_sb = pers.tile([P, P], f32)
            mask2_sb = pers.tile([P, 2 * P], f32)
            bq_sb = pers.tile([P, CGP], f32)
            bk_sb = pers.tile([P, CGP], f32)
            ones_sb = pers.tile([P, HG], f32)

            nc.sync.dma_start(out=wo_sb, in_=wo_d.ap().rearrange("(c p) n -> p c n", p=P))
            nc.sync.dma_start(out=mask_sb, in_=mask_d.ap())
            nc.sync.dma_start(out=mask2_sb, in_=mask2_d.ap())
            nc.sync.dma_start(out=bq_sb, in_=bq_d.ap())
            nc.sync.dma_start(out=bk_sb, in_=bk_d.ap())
            nc.vector.memset(ones_sb, 1.0)
            # zero halves of kz that the k-projection copies never touch
            nc.vector.memset(kz[0:HD, :, 1, :], 0.0)
            nc.vector.memset(kz[HD:P, :, 0, :], 0.0)

            # ---- phase B: projections, two half-passes over c_in ----
            with (
                tc.tile_pool(name="xchunk", bufs=6) as xpool,
                tc.tile_pool(name="wchunk", bufs=2) as wpool,
            ):
                for half in range(2):
                    xc = []
                    for i in range(4):
                        ci = half * 4 + i
                        t_ = xpool.tile([P, T], f32r, tag="xc")
                        nc.sync.dma_start(out=t_, in_=xt_d.ap()[ci * P:(ci + 1) * P, :])
                        xc.append(t_)
                    wh = wpool.tile([P, 3, 4, CG], f32r)
                    for wi, wd in enumerate((wq_d, wk_d, wv_d)):
                        nc.sync.dma_start(
                            out=wh[:, wi, :, :],
                            in_=wd.ap()[half * CG:(half + 1) * CG, :].rearrange(
                                "(c p) n -> p c n", p=P
                            ),
                        )
                    def emit_qk(j, w, half=half, xc=xc, wh=wh):
                        ws = slice(w * W, (w + 1) * W)
                        # q
                        ps = psum_mm.tile([P, W], f32, tag="ps")
                        for i in range(4):
                            mm(ps, wh[:, 0, i, j * P:(j + 1) * P],
                               xc[i][:, ws], start=(i == 0), stop=(i == 3))
                        if half == 0:
                            nc.scalar.activation(qT[:, j, ws], ps,
                                                 AF.Identity, bias=bq_sb[:, j:j + 1])
                        else:
                            nc.vector.tensor_add(qT[:, j, ws], qT[:, j, ws], ps)
                        # k -> zero-padded kz layout
                        ps = psum_mm.tile([P, W], f32, tag="ps")
                        for i in range(4):
                            mm(ps, wh[:, 1, i, j * P:(j + 1) * P],
                               xc[i][:, ws], start=(i == 0), stop=(i == 3))
                        for a in range(2):
                            rows = slice(a * HD, (a + 1) * HD)
                            dst = kz[rows, j, a, ws]
                            if half == 0:
                                nc.scalar.activation(dst, ps[rows, :], AF.Identity,
                                                     bias=bk_sb[rows, j:j + 1])
                            else:
                                nc.vector.tensor_add(dst, dst, ps[rows, :])

                    def emit_v(it, half=half, xc=xc, wh=wh):
                        # v in row layout [t, c_out], strided into vp per head
                        ps = psum_mm.tile([P, CG], f32, tag="ps")
                        for i in range(4):
                            mm(ps, xc[i][:, it * P:(it + 1) * P], wh[:, 2, i, :],
                               start=(i == 0), stop=(i == 3))
                        v_view = vp[:, it, :].rearrange("p (h x) -> p h x", x=VS)[:, :, 0:HD]
                        ps_view = ps.rearrange("p (h x) -> p h x", x=HD)
                        if half == 0:
                            nc.scalar.copy(v_view, ps_view)
                        else:
                            nc.vector.tensor_add(v_view, v_view, ps_view)

                    if half == 0:
                        for j in range(CGP):
                            for w in range(NW):
                                emit_qk(j, w)
                        for it in range(NTT):
                            emit_v(it)
                    else:
                        # half 1 ordered by window so early attention windows
                        # can start while the projection tail still runs
                        for w in range(NW):
                            for it in range(4 * w, 4 * w + 4):
                                emit_v(it)
                            for j in range(CGP):
                                emit_qk(j, w)
            for it in range(NTT):
                ones_view = vp[:, it, :].rearrange("p (h x) -> p h x", x=VS)[:, :, HD:VS]
                nc.vector.tensor_copy(
                    ones_view, ones_sb.rearrange("p (h x) -> p h x", x=1))

            # ---- phase C+D: attention with interleaved output projection ----
            with (
                tc.tile_pool(name="attn", bufs=1) as attn,
                tc.tile_pool(name="pt", bufs=6) as ptpool,
                tc.tile_pool(name="dn", bufs=2) as dnpool,
                tc.tile_pool(name="rb", bufs=2) as rbpool,
                tc.tile_pool(name="psum_av", bufs=3, space="PSUM") as psum_av,
                tc.tile_pool(name="osb", bufs=3) as opool,
                tc.tile_pool(name="psum_o", bufs=1, space="PSUM") as psum_o,
            ):
                yT = attn.tile([P, CGP, T], f32r)
                for w in range(NW):
                    tq0 = w * W
                    for h in range(HG):
                        j, a = h // 2, h % 2
                        ps_av = psum_av.tile([VS, W], f32, tag="av")
                        ntk = (w + 1) * (W // P)
                        escale = 1.0 / float(np.sqrt(HD))
                        # tk blocks go in pairs sharing one psum pair-tile and
                        # one exp; a diagonal second block is extended to the
                        # pair's region and cleaned up by mask2 (128 zero cols
                        # + 128 triangular cols)
                        for x in range(ntk // 2):
                            i0, i1 = 2 * x, 2 * x + 1
                            vs0 = max(tq0, i0 * P)
                            n0 = tq0 + W - vs0
                            ps_s = psum_mm.tile([P, 2, W], f32, tag="ps")
                            pt = ptpool.tile([P, 2, W], f32r, tag="pt")
                            mm(ps_s[:, 0, W - n0:], kz[:, j, a, i0 * P:(i0 + 1) * P],
                               qT[:, j, vs0:vs0 + n0], start=True, stop=True)
                            mm(ps_s[:, 1, W - n0:], kz[:, j, a, i1 * P:(i1 + 1) * P],
                               qT[:, j, vs0:vs0 + n0], start=True, stop=True)
                            nc.scalar.activation(pt[:, :, W - n0:], ps_s[:, :, W - n0:],
                                                 AF.Exp, scale=escale)
                            if i0 * P >= tq0:  # diagonal pair
                                nc.vector.tensor_mul(
                                    pt[:, 0, W - n0:W - n0 + P],
                                    pt[:, 0, W - n0:W - n0 + P], mask_sb)
                                nc.vector.tensor_mul(
                                    pt[:, 1, W - n0:W - n0 + 2 * P],
                                    pt[:, 1, W - n0:W - n0 + 2 * P], mask2_sb)
                            mm(ps_av[:, vs0 - tq0:], vp[:, i0, h * VS:(h + 1) * VS],
                               pt[:, 0, W - n0:], start=(i0 == 0), stop=False)
                            mm(ps_av[:, vs0 - tq0:], vp[:, i1, h * VS:(h + 1) * VS],
                               pt[:, 1, W - n0:], start=False, stop=(i1 == ntk - 1))
                        dn = dnpool.tile([1, W], f32, tag="dn")
                        nc.vector.tensor_copy(dn, ps_av[HD:VS, :])
                        rb = rbpool.tile([HD, W], f32, tag="rb")
                        nc.gpsimd.partition_broadcast(rb, dn)
                        nc.vector.reciprocal_approx_fast(out=rb, in_=rb)
                        nc.vector.tensor_mul(
                            yT[a * HD:(a + 1) * HD, j, tq0:tq0 + W],
                            ps_av[0:HD, :], rb)
                    # output projection for this window
                    for m in range(C // P):
                        po = psum_o.tile([P, W], f32, tag="po")
                        for i in range(CGP):
                            mm(po, wo_sb[:, i, m * P:(m + 1) * P],
                               yT[:, i, tq0:tq0 + W],
                               start=(i == 0), stop=(i == CGP - 1))
                        ot = opool.tile([P, W], f32, tag="ot")
                        nc.vector.tensor_copy(ot, po)
                        nc.sync.dma_start(
                            out=out_d.ap()[m * P:(m + 1) * P, tq0:tq0 + W],
                            in_=ot)

    nc.compile()
    return nc


def get_nc():
    global _cached_nc
    if _cached_nc is None:
        _cached_nc = _build()
    return _cached_nc


def make_in_maps(x, Wq, bq, Wk, bk, Wv, bv, Wo, bo):
    x = np.asarray(x, np.float32)
    mask = np.triu(np.ones((P, P), np.float32))
    mask2 = np.concatenate([np.zeros((P, P), np.float32), mask], axis=1)
    in_maps = []
    for c in range(8):
        b, g = c // 2, c % 2
        cs = slice(g * CG, (g + 1) * CG)
        in_maps.append({
            "xt": np.ascontiguousarray(x[b].T),
            "wq": np.ascontiguousarray(np.asarray(Wq, np.float32)[:, cs]),
            "wk": np.ascontiguousarray(np.asarray(Wk, np.float32)[:, cs]),
            "wv": np.ascontiguousarray(np.asarray(Wv, np.float32)[:, cs]),
            "wo": np.ascontiguousarray(np.asarray(Wo, np.float32)[cs, :]),
            "bq": np.ascontiguousarray(
                np.asarray(bq, np.float32)[cs].reshape(CGP, P).T),
            "bk": np.ascontiguousarray(
                np.asarray(bk, np.float32)[cs].reshape(CGP, P).T),
            "mask": mask,
            "mask2": mask2,
        })
    return in_maps


def combine(results, Wv, bv, Wo, bo):
    const = (np.asarray(bv, np.float32) @ np.asarray(Wo, np.float32)
             + np.asarray(bo, np.float32))
    out = np.empty((B, T, C), np.float32)
    for b in range(B):
        acc = results[2 * b]["outp"] + results[2 * b + 1]["outp"]
        out[b] = acc.T + const[None, :]
    return out


def kernel(x, Wq, bq, Wk, bk, Wv, bv, Wo, bo):
    nc = get_nc()
    in_maps = make_in_maps(x, Wq, bq, Wk, bk, Wv, bv, Wo, bo)
    res = run_bass_kernel_spmd(nc, in_maps, core_ids=list(range(8)))
    return combine(res.results, Wv, bv, Wo, bo)


# revision 15
# speedup vs baseline: 1.4200x; 1.0307x over previous
"""Causal self-attention on 8 trn2 NeuronCores.

Sharding: core c -> (batch b = c//2, head-group g = c%2).  Each head-group
is 8 heads = 512 channels.  Per core:
  - q/k/v projections of x[b] restricted to the group's 512 columns
  - causal attention for the 8 heads, computed in the transposed
    orientation S^T = [tk, tq] so softmax denominators come from a
    ones-column appended to V (AV matmul yields them for free) and no
    transposes are needed anywhere
  - partial output projection through the group's 512 rows of Wo
Host sums the two partials per batch and adds (bv @ Wo + bo): softmax
weights sum to one, so the v-bias passes through attention additively.

The QK stationary operand is zero-padded to a full [128,128] footprint
(kz holds each head's k tile in its 64 q-channel rows, zeros in the other
head's rows, so the full 128-partition q pair streams through) — without
this the PE activity monitor keeps the tensor engine clock-gated at
1.2 GHz for the whole attention phase.
"""

import numpy as np
import ml_dtypes

import concourse.bass as bass
import concourse.mybir as mybir
from concourse import bacc, tile
from concourse.bass_utils import run_bass_kernel_spmd

B, T, C, H = 4, 2048, 1024, 16
HD = C // H          # 64
G = 2                # head groups (cores per batch)
HG = H // G          # 8 heads per group
CG = C // G          # 512 channels per group
CGP = CG // 128      # 4 c_out tiles per group
P = 128
W = 512              # free-dim window (one PSUM bank of f32)
NW = T // W          # 4 windows
NTT = T // P         # 16 t tiles
VS = HD + 1          # 65: v plus ones column

_cached_nc = None


def _build():
    f32 = mybir.dt.float32
    f32r = mybir.dt.float32r
    bf16 = mybir.dt.bfloat16
    AF = mybir.ActivationFunctionType
    nc = bacc.Bacc("TRN2", target_bir_lowering=False, debug=False, num_devices=8)

    xt_d = nc.dram_tensor("xt", [C, T], f32r, kind="ExternalInput")
    wq_d = nc.dram_tensor("wq", [C, CG], f32r, kind="ExternalInput")
    wk_d = nc.dram_tensor("wk", [C, CG], f32r, kind="ExternalInput")
    wv_d = nc.dram_tensor("wv", [C, CG], f32r, kind="ExternalInput")
    wo_d = nc.dram_tensor("wo", [CG, C], f32r, kind="ExternalInput")
    bq_d = nc.dram_tensor("bq", [P, CGP], f32, kind="ExternalInput")
    bk_d = nc.dram_tensor("bk", [P, CGP], f32, kind="ExternalInput")
    mask_d = nc.dram_tensor("mask", [P, P], f32, kind="ExternalInput")
    mask2_d = nc.dram_tensor("mask2", [P, 2 * P], f32, kind="ExternalInput")
    out_d = nc.dram_tensor("outp", [C, T], f32, kind="ExternalOutput")

    mm = lambda out, lhsT, rhs, start, stop: nc.tensor.matmul(
        out, lhsT, rhs, start=start, stop=stop)

    with tile.TileContext(nc) as tc:
        with (
            tc.tile_pool(name="pers", bufs=1) as pers,
            tc.tile_pool(name="psum_mm", bufs=2, space="PSUM") as psum_mm,
        ):
            qT = pers.tile([P, CGP, T], bf16)        # q^T: [c_out, t]
            # kz[:, j, a, tk]: head h=2j+a k-tile in its own 64 rows, 0 else
            kz = pers.tile([P, CGP, 2, T], bf16)
            vp = pers.tile([P, NTT, HG * VS], f32r)  # v rows + ones col/head
            wo_sb = pers.tile([P, CGP, C], f32r)
            mask_sb = pers.tile([P, P], f32)
            mask2_sb = pers.tile([P, 2 * P], f32)
            bq_sb = pers.tile([P, CGP], f32)
            bk_sb = pers.tile([P, CGP], f32)
            ones_sb = pers.tile([P, HG], f32)

            nc.sync.dma_start(out=wo_sb, in_=wo_d.ap().rearrange("(c p) n -> p c n", p=P))
            nc.sync.dma_start(out=mask_sb, in_=mask_d.ap())
            nc.sync.dma_start(out=mask2_sb, in_=mask2_d.ap())
            nc.sync.dma_start(out=bq_sb, in_=bq_d.ap())
            nc.sync.dma_start(out=bk_sb, in_=bk_d.ap())
            nc.vector.memset(ones_sb, 1.0)
            # zero halves of kz that the k-projection copies never touch
            nc.vector.memset(kz[0:HD, :, 1, :], 0.0)
            nc.vector.memset(kz[HD:P, :, 0, :], 0.0)

            # ---- phase B: projections, two half-passes over c_in ----
            with (
                tc.tile_pool(name="xchunk", bufs=6) as xpool,
                tc.tile_pool(name="wchunk", bufs=2) as wpool,
            ):
                for half in range(2):
                    wh = wpool.tile([P, 3, 4, CG], f32r)
                    for wi, wd in enumerate((wq_d, wk_d, wv_d)):
                        nc.sync.dma_start(
                            out=wh[:, wi, :, :],
                            in_=wd.ap()[half * CG:(half + 1) * CG, :].rearrange(
                                "(c p) n -> p c n", p=P
                            ),
                        )
                    xc = []
                    for i in range(4):
                        ci = half * 4 + i
                        t_ = xpool.tile([P, T], f32r, tag="xc")
                        # two half-tile DMAs so the first matmuls start sooner
                        nc.sync.dma_start(out=t_[:, 0:T // 2],
                                          in_=xt_d.ap()[ci * P:(ci + 1) * P, 0:T // 2])
                        nc.sync.dma_start(out=t_[:, T // 2:],
                                          in_=xt_d.ap()[ci * P:(ci + 1) * P, T // 2:])
                        xc.append(t_)
                    def emit_qk(j, w, half=half, xc=xc, wh=wh):
                        ws = slice(w * W, (w + 1) * W)
                        # q
                        ps = psum_mm.tile([P, W], f32, tag="ps")
                        for i in range(4):
                            mm(ps, wh[:, 0, i, j * P:(j + 1) * P],
                               xc[i][:, ws], start=(i == 0), stop=(i == 3))
                        if half == 0:
                            nc.scalar.activation(qT[:, j, ws], ps,
                                                 AF.Identity, bias=bq_sb[:, j:j + 1])
                        else:
                            nc.vector.tensor_add(qT[:, j, ws], qT[:, j, ws], ps)
                        # k -> zero-padded kz layout
                        ps = psum_mm.tile([P, W], f32, tag="ps")
                        for i in range(4):
                            mm(ps, wh[:, 1, i, j * P:(j + 1) * P],
                               xc[i][:, ws], start=(i == 0), stop=(i == 3))
                        for a in range(2):
                            rows = slice(a * HD, (a + 1) * HD)
                            dst = kz[rows, j, a, ws]
                            if half == 0:
                                nc.scalar.activation(dst, ps[rows, :], AF.Identity,
                                                     bias=bk_sb[rows, j:j + 1])
                            else:
                                nc.vector.tensor_add(dst, dst, ps[rows, :])

                    def emit_v(it, half=half, xc=xc, wh=wh):
                        # v in row layout [t, c_out], strided into vp per head
                        ps = psum_mm.tile([P, CG], f32, tag="ps")
                        for i in range(4):
                            mm(ps, xc[i][:, it * P:(it + 1) * P], wh[:, 2, i, :],
                               start=(i == 0), stop=(i == 3))
                        v_view = vp[:, it, :].rearrange("p (h x) -> p h x", x=VS)[:, :, 0:HD]
                        ps_view = ps.rearrange("p (h x) -> p h x", x=HD)
                        if half == 0:
                            nc.scalar.copy(v_view, ps_view)
                        else:
                            nc.vector.tensor_add(v_view, v_view, ps_view)

                    if half == 0:
                        for j in range(CGP):
                            for w in range(NW):
                                emit_qk(j, w)
                        for it in range(NTT):
                            emit_v(it)
                    else:
                        # half 1 ordered by window so early attention windows
                        # can start while the projection tail still runs
                        for w in range(NW):
                            for it in range(4 * w, 4 * w + 4):
                                emit_v(it)
                            for j in range(CGP):
                                emit_qk(j, w)
            for it in range(NTT):
                ones_view = vp[:, it, :].rearrange("p (h x) -> p h x", x=VS)[:, :, HD:VS]
                nc.vector.tensor_copy(
                    ones_view, ones_sb.rearrange("p (h x) -> p h x", x=1))

            # ---- phase C+D: attention with interleaved output projection ----
            with (
                tc.tile_pool(name="attn", bufs=1) as attn,
                tc.tile_pool(name="pt", bufs=6) as ptpool,
                tc.tile_pool(name="dn", bufs=2) as dnpool,
                tc.tile_pool(name="rb", bufs=2) as rbpool,
                tc.tile_pool(name="psum_av", bufs=2, space="PSUM") as psum_av,
                tc.tile_pool(name="osb", bufs=3) as opool,
                tc.tile_pool(name="psum_o", bufs=2, space="PSUM") as psum_o,
            ):
                yT = attn.tile([P, CGP, T], f32r)
                for w in range(NW):
                    tq0 = w * W
                    for h in range(HG):
                        j, a = h // 2, h % 2
                        ps_av = psum_av.tile([VS, W], f32, tag="av")
                        ntk = (w + 1) * (W // P)
                        escale = 1.0 / float(np.sqrt(HD))
                        # tk blocks go in pairs sharing one psum pair-tile and
                        # one exp; a diagonal second block is extended to the
                        # pair's region and cleaned up by mask2 (128 zero cols
                        # + 128 triangular cols)
                        for x in range(ntk // 2):
                            i0, i1 = 2 * x, 2 * x + 1
                            vs0 = max(tq0, i0 * P)
                            n0 = tq0 + W - vs0
                            ps_s = psum_mm.tile([P, 2, W], f32, tag="ps")
                            pt = ptpool.tile([P, 2, W], f32r, tag="pt")
                            mm(ps_s[:, 0, W - n0:], kz[:, j, a, i0 * P:(i0 + 1) * P],
                               qT[:, j, vs0:vs0 + n0], start=True, stop=True)
                            mm(ps_s[:, 1, W - n0:], kz[:, j, a, i1 * P:(i1 + 1) * P],
                               qT[:, j, vs0:vs0 + n0], start=True, stop=True)
                            nc.scalar.activation(pt[:, :, W - n0:], ps_s[:, :, W - n0:],
                                                 AF.Exp, scale=escale)
                            if i0 * P >= tq0:  # diagonal pair
                                nc.vector.tensor_mul(
                                    pt[:, 0, W - n0:W - n0 + P],
                                    pt[:, 0, W - n0:W - n0 + P], mask_sb)
                                nc.vector.tensor_mul(
                                    pt[:, 1, W - n0:W - n0 + 2 * P],
                                    pt[:, 1, W - n0:W - n0 + 2 * P], mask2_sb)
                            mm(ps_av[:, vs0 - tq0:], vp[:, i0, h * VS:(h + 1) * VS],
                               pt[:, 0, W - n0:], start=(i0 == 0), stop=False)
                            mm(ps_av[:, vs0 - tq0:], vp[:, i1, h * VS:(h + 1) * VS],
                               pt[:, 1, W - n0:], start=False, stop=(i1 == ntk - 1))
                        dn = dnpool.tile([1, W], f32, tag="dn")
                        nc.vector.tensor_copy(dn, ps_av[HD:VS, :])
                        rb = rbpool.tile([HD, W], f32, tag="rb")
                        nc.gpsimd.partition_broadcast(rb, dn)
                        nc.vector.reciprocal_approx_fast(out=rb, in_=rb)
                        nc.vector.tensor_mul(
                            yT[a * HD:(a + 1) * HD, j, tq0:tq0 + W],
                            ps_av[0:HD, :], rb)
                    # output projection for this window
                    for m in range(C // P):
                        po = psum_o.tile([P, W], f32, tag="po")
                        for i in range(CGP):
                            mm(po, wo_sb[:, i, m * P:(m + 1) * P],
                               yT[:, i, tq0:tq0 + W],
                               start=(i == 0), stop=(i == CGP - 1))
                        ot = opool.tile([P, W], f32, tag="ot")
                        nc.vector.tensor_copy(ot, po)
                        nc.sync.dma_start(
                            out=out_d.ap()[m * P:(m + 1) * P, tq0:tq0 + W],
                            in_=ot)

    nc.compile()
    return nc


def get_nc():
    global _cached_nc
    if _cached_nc is None:
        _cached_nc = _build()
    return _cached_nc


def make_in_maps(x, Wq, bq, Wk, bk, Wv, bv, Wo, bo):
    x = np.asarray(x, np.float32)
    mask = np.triu(np.ones((P, P), np.float32))
    mask2 = np.concatenate([np.zeros((P, P), np.float32), mask], axis=1)
    in_maps = []
    for c in range(8):
        b, g = c // 2, c % 2
        cs = slice(g * CG, (g + 1) * CG)
        in_maps.append({
            "xt": np.ascontiguousarray(x[b].T),
            "wq": np.ascontiguousarray(np.asarray(Wq, np.float32)[:, cs]),
            "wk": np.ascontiguousarray(np.asarray(Wk, np.float32)[:, cs]),
            "wv": np.ascontiguousarray(np.asarray(Wv, np.float32)[:, cs]),
            "wo": np.ascontiguousarray(np.asarray(Wo, np.float32)[cs, :]),
            "bq": np.ascontiguousarray(
                np.asarray(bq, np.float32)[cs].reshape(CGP, P).T),
            "bk": np.ascontiguousarray(
                np.asarray(bk, np.float32)[cs].reshape(CGP, P).T),
            "mask": mask,
            "mask2": mask2,
        })
    return in_maps


def combine(results, Wv, bv, Wo, bo):
    const = (np.asarray(bv, np.float32) @ np.asarray(Wo, np.float32)
             + np.asarray(bo, np.float32))
    out = np.empty((B, T, C), np.float32)
    for b in range(B):
        acc = results[2 * b]["outp"] + results[2 * b + 1]["outp"]
        out[b] = acc.T + const[None, :]
    return out


def kernel(x, Wq, bq, Wk, bk, Wv, bv, Wo, bo):
    nc = get_nc()
    in_maps = make_in_maps(x, Wq, bq, Wk, bk, Wv, bv, Wo, bo)
    res = run_bass_kernel_spmd(nc, in_maps, core_ids=list(range(8)))
    return combine(res.results, Wv, bv, Wo, bo)
